# revision 29
# baseline (speedup 1.0000x reference)
"""FFM layer kernel for Trainium2 (8 NeuronCores, data-parallel over batch).

Math (reference):
  idx[b,j]  = 13 + j*10000 + sparse_x[b,j]                 (26 sparse fields)
  linear    = dense_x @ w[:13] + sum_j w[idx] + w0         (B,1)
  field_f   = einsum('bd,dfk', dense_x, v[:13]) + sum_j v[idx]   (B,39,8)
  s         = sum_f field_f                                 (B,8)
  cross     = 0.5*(sum_k s^2 - sum_{f,k} field_f^2)
  out       = sigmoid(linear + cross)

Device strategy (per core, 512 samples): fp16 padded table rows (384 f16 =
768B) carrying [v row | w | s_row[k]=sum_f v[.,f,k]], 768-idx two-field
gathers (int16 idx spans adjacent vocab blocks via +10000 offsets) balanced
3328 rows per SWDGE queue in 5 round-robin rounds (gen rate ~ drain rate,
quarters last), two interleaved fp16 accumulator chains on DVE plus an
fp32 chain for the high-magnitude s columns, PE matmul for the dense part,
DVE/ACT final phase with s read from the precomputed table columns.
"""

import os
import numpy as np

N_DENSE = 13
N_SPARSE = 26
VOCAB = 10000
N_FIELD = 39
N_FEAT = N_DENSE + N_SPARSE * VOCAB  # 260013
K = 8
ROW = N_FIELD * K  # 312 v elems; w at col 312; s_row at cols 313-320
USED = ROW + 1  # 313 cols accumulated in fp16 (v + w)
SCOL = ROW + 1  # first s column
ROWE = 384  # padded fp16 row -> 768 B (%256==0)
BATCH = 4096
N_CORES = 8
BC = BATCH // N_CORES  # 512 per core
P = 128
NCHUNK = BC // P  # 4
IDXC = BC // 16  # 32 int16 index columns per field
NQ = 4  # SWDGE queues / GPSIMD core pairs

# Per-queue gather schedule over the global idx sequence (field-major,
# position i = field*512 + sample).  Uniform 512-row single-field gathers:
# Q7 descriptor gen (~4.4us/round of 4) matches the HBM drain rate of a
# round (4*512*768B at ~360GB/s = 4.4us), so with single_packet doorbells
# (drain follows own gen) the rounds pipeline hole-free at the HBM roof.
# The two half-fields (6, 19) are 256-row gathers scheduled LAST so the
# final exposed drain is small.  Entry: (field_window_start, idx_lo, idx_hi).
SCHED = [
    [(0, 0, 768), (1, 768, 1536), (3, 1536, 2304), (4, 2304, 3072),
     (6, 3072, 3328)],
    [(7, 3584, 4352), (8, 4352, 5120), (10, 5120, 5888), (11, 5888, 6656),
     (6, 3328, 3584)],
    [(13, 6656, 7424), (14, 7424, 8192), (16, 8192, 8960), (17, 8960, 9728),
     (19, 9728, 9984)],
    [(20, 10240, 11008), (21, 11008, 11776), (23, 11776, 12544),
     (24, 12544, 13312), (19, 9984, 10240)],
]


def _segments(w, lo, hi):
    """Yield (tile_chunk0, acc_chunk0, n_chunks, field) per field segment."""
    segs = []
    f = lo // BC
    while f * BC < hi:
        seg_lo = max(lo, f * BC)
        seg_hi = min(hi, (f + 1) * BC)
        segs.append(((seg_lo - lo) // P, (seg_lo - f * BC) // P,
                     (seg_hi - seg_lo) // P, f))
        f += 1
    return segs

_CACHE: dict = {}


def _build_program():
    import concourse.bacc as bacc
    import concourse.tile as tile
    import concourse.mybir as mybir

    f32 = mybir.dt.float32
    f16 = mybir.dt.float16
    i16 = mybir.dt.int16

    nc = bacc.Bacc(
        "TRN2", target_bir_lowering=False, debug=False, num_swdge_queues=NQ
    )

    table = nc.dram_tensor("table", [N_FEAT, ROWE], f16, kind="ExternalInput")
    xt = nc.dram_tensor("xt", [P, BC], f32, kind="ExternalInput")
    vd = nc.dram_tensor("vd", [P, ROWE], f32, kind="ExternalInput")
    idx16 = nc.dram_tensor("idx16", [P, N_SPARSE * IDXC], i16, kind="ExternalInput")
    # out[p, c] = sigmoid result for sample c*128+p; host transposes
    out = nc.dram_tensor("out", [P, NCHUNK], f32, kind="ExternalOutput")

    with tile.TileContext(nc) as tc:
        with (
            tc.tile_pool(name="const", bufs=1) as cpool,
            tc.tile_pool(name="gather", bufs=4 * NQ) as gpool,
            tc.tile_pool(name="work", bufs=1) as wpool,
            tc.tile_pool(name="psum", bufs=1, space="PSUM") as ppool,
        ):
            # index tiles first: the gathers depend only on these
            idx_sb = cpool.tile([P, N_SPARSE * IDXC], i16)
            nc.scalar.dma_start(out=idx_sb[:], in_=idx16[:])
            # dense inputs on the other HWDGE queue; only the PE needs them
            xt_sb = cpool.tile([P, BC], f32)
            nc.sync.dma_start(out=xt_sb[:], in_=xt[:])
            vd_sb = cpool.tile([P, ROWE], f32)
            nc.sync.dma_start(out=vd_sb[:], in_=vd[:])

            # preload the ACT sigmoid table off the critical tail
            warm = cpool.tile([P, 1], f32)
            nc.vector.memset(warm[:], 0.0)
            warm2 = cpool.tile([P, 1], f32)
            nc.scalar.activation(
                warm2[:], warm[:], mybir.ActivationFunctionType.Sigmoid
            )

            # dense part: one matmul per chunk, each into its own PSUM bank
            psum = ppool.tile([P, NCHUNK, 512], f32, space="PSUM")
            for c in range(NCHUNK):
                nc.tensor.matmul(
                    out=psum[:, c, :ROWE],
                    lhsT=xt_sb[:, c * P:(c + 1) * P],
                    rhs=vd_sb[:],
                    start=True,
                    stop=True,
                )

            nregs = {}
            for q in range(NQ):
                for _w, lo, hi in SCHED[q]:
                    ni = hi - lo
                    if ni not in nregs:
                        nregs[ni] = nc.gpsimd.to_reg(ni)
            # emit gathers round-robin across queues; the first gather also
            # absorbs the one-time ~9us GPSIMD ext-isa IRAM load
            gathers = []  # (tile, queue, window_first, lo, hi)
            for r in range(max(len(s) for s in SCHED)):
                for q in range(NQ):
                    if r >= len(SCHED[q]):
                        continue
                    w, lo, hi = SCHED[q][r]
                    ni = hi - lo
                    nch = ni // P
                    last_f = (hi - 1) // BC
                    g = gpool.tile([P, nch, ROWE], f16, tag="g", name=f"g{q}_{r}")
                    base = N_DENSE + w * VOCAB
                    nc.gpsimd.dma_gather(
                        out_ap=g[:],
                        in_ap=table[base:base + (last_f - w + 1) * VOCAB, :],
                        idxs_ap=idx_sb[:, lo // 16:hi // 16],
                        num_idxs=ni,
                        num_idxs_reg=nregs[ni],
                        elem_size=ROWE,
                        single_packet=True,
                        queue_num=q,
                    )
                    gathers.append((g, q, w, lo, hi))

            # two fp16 accumulator chains with alternating segment assignment
            # (adjacent DVE adds hit different chains so they pipeline) plus
            # a flat fp32 chain for the high-magnitude s columns (313-320).
            # memset inits run during the startup dead time.
            accs = [wpool.tile([P, NCHUNK, USED + 1], f16, name=f"acc{i}")
                    for i in range(2)]
            sacc2 = wpool.tile([P, 2 * NCHUNK, K], f32)
            for a in accs:
                nc.vector.memset(a[:], 0.0)
            nc.vector.memset(sacc2[:], 0.0)

            nseg = 0
            for t, (g, q, w, lo, hi) in enumerate(gathers):
                for tc0, ac0, nch, f in _segments(w, lo, hi):
                    a = accs[nseg % 2][:, ac0:ac0 + nch, :]
                    nc.vector.tensor_tensor(
                        out=a, in0=a,
                        in1=g[:, tc0:tc0 + nch, :USED + 1],
                        op=mybir.AluOpType.add,
                    )
                    nseg += 1
                # one flat fp32 s-column add per gather: within a tile,
                # acc_chunk - tile_chunk == (lo/128) mod 4, so slot base
                # x = (lo/128)%4 keeps slot % 4 == chunk for every sample
                x = (lo // P) % NCHUNK
                nch_t = (hi - lo) // P
                sd = sacc2[:, x:x + nch_t, :]
                nc.vector.tensor_tensor(
                    out=sd, in0=sd, in1=g[:, :, SCOL:SCOL + K],
                    op=mybir.AluOpType.add,
                )

            # combine the two chains, then field = psum + acc
            nc.vector.tensor_tensor(out=accs[0][:], in0=accs[0][:],
                                    in1=accs[1][:], op=mybir.AluOpType.add)
            field = wpool.tile([P, NCHUNK, 320], f32)
            nc.vector.tensor_tensor(
                out=field[:, :, :USED], in0=psum[:, :, :USED],
                in1=accs[0][:, :, :USED], op=mybir.AluOpType.add,
            )

            # q = sum(field[:, :, :312]^2): ACT squares while the DVE runs
            # the small s ops below, then reduces
            sq = wpool.tile([P, NCHUNK, ROW], f32)
            nc.scalar.square(sq[:], field[:, :, :ROW])

            # s = dense part (psum cols 313-320, from vd s columns) + both
            # halves of the flat fp32 s accumulator; ssum = sum_k s^2
            s_t = wpool.tile([P, NCHUNK, K], f32)
            nc.vector.tensor_tensor(
                out=s_t[:], in0=sacc2[:, 0:NCHUNK, :],
                in1=sacc2[:, NCHUNK:2 * NCHUNK, :], op=mybir.AluOpType.add,
            )
            s = wpool.tile([P, NCHUNK, K], f32)
            nc.vector.tensor_tensor(
                out=s[:], in0=psum[:, :, SCOL:SCOL + K], in1=s_t[:],
                op=mybir.AluOpType.add,
            )
            ss = wpool.tile([P, NCHUNK, K], f32)
            nc.vector.tensor_tensor(out=ss[:], in0=s[:], in1=s[:],
                                    op=mybir.AluOpType.mult)
            ssum = wpool.tile([P, NCHUNK, 1], f32)
            nc.vector.reduce_sum(out=ssum[:], in_=ss[:], axis=mybir.AxisListType.X)

            qs = wpool.tile([P, NCHUNK, 1], f32)
            nc.vector.reduce_sum(out=qs[:], in_=sq[:], axis=mybir.AxisListType.X)

            d = wpool.tile([P, NCHUNK, 1], f32)
            nc.vector.tensor_tensor(out=d[:], in0=ssum[:], in1=qs[:],
                                    op=mybir.AluOpType.subtract)
            # dd = 0.5*d + linear
            dd = wpool.tile([P, NCHUNK, 1], f32)
            nc.vector.scalar_tensor_tensor(
                out=dd[:], in0=d[:], scalar=0.5, in1=field[:, :, ROW:ROW + 1],
                op0=mybir.AluOpType.mult, op1=mybir.AluOpType.add,
            )
            oc = wpool.tile([P, NCHUNK], f32)
            nc.scalar.activation(
                oc[:], dd[:, :, 0], mybir.ActivationFunctionType.Sigmoid
            )
            nc.sync.dma_start(out=out[:], in_=oc[:])

    nc.compile()
    return nc


def _prep_inputs(dense_x, sparse_x, w0, w, v):
    table = np.zeros((N_FEAT, ROWE), dtype=np.float16)
    table[:, :ROW] = v.reshape(N_FEAT, ROW).astype(np.float16)
    table[:, ROW] = w[:, 0].astype(np.float16)
    # per-k field sums: s_row[i, k] = sum_f v[i, f, k]
    table[:, SCOL:SCOL + K] = v.sum(axis=1).astype(np.float16)

    vd = np.zeros((P, ROWE), dtype=np.float32)
    vd[:N_DENSE, :ROW] = v[:N_DENSE].reshape(N_DENSE, ROW)
    vd[:N_DENSE, ROW] = w[:N_DENSE, 0]
    vd[N_DENSE, ROW] = np.float32(w0[0])
    vd[:N_DENSE, SCOL:SCOL + K] = v[:N_DENSE].sum(axis=1)

    xt_full = np.zeros((P, BATCH), dtype=np.float32)
    xt_full[:N_DENSE] = dense_x.T
    xt_full[N_DENSE] = 1.0

    # per-column idx value offsets: global position i = col*16 + row lives in
    # field i//512; its gather window starts at field w -> offset (f-w)*VOCAB
    col_off = np.zeros(N_SPARSE * IDXC, dtype=np.int16)
    for qsched in SCHED:
        for w, lo, hi in qsched:
            for col in range(lo // 16, hi // 16):
                f = (col * 16) // BC
                col_off[col] = (f - w) * VOCAB

    in_maps = []
    for r in range(N_CORES):
        b0 = r * BC
        sp = sparse_x[b0:b0 + BC].astype(np.int16)  # values < 10000 fit
        idx16 = np.zeros((P, N_SPARSE * IDXC), dtype=np.int16)
        for j in range(N_SPARSE):
            blk = sp[:, j].reshape(IDXC, 16).T
            idx16[:, j * IDXC:(j + 1) * IDXC] = np.tile(blk, (P // 16, 1))
        idx16 += col_off[None, :]
        in_maps.append(
            {
                "table": table,
                "xt": np.ascontiguousarray(xt_full[:, b0:b0 + BC]),
                "vd": vd,
                "idx16": idx16,
            }
        )
    return in_maps


def kernel(dense_x, sparse_x, w0, w, v, _trace=False, _trace_kwargs=None):
    from concourse.bass_utils import run_bass_kernel_spmd

    if "nc" not in _CACHE:
        _CACHE["nc"] = _build_program()
    nc = _CACHE["nc"]

    in_maps = _prep_inputs(dense_x, sparse_x, w0, w, v)
    kw = {}
    if _trace:
        kw["trace"] = True
        if _trace_kwargs:
            kw.update(_trace_kwargs)
    res = run_bass_kernel_spmd(nc, in_maps, core_ids=list(range(N_CORES)), **kw)
    # device out[p, c] holds sample c*128+p of the core's 512-sample slice
    outs = [res.results[r]["out"].T.reshape(BC, 1) for r in range(N_CORES)]
    full = np.concatenate(outs, axis=0).astype(np.float32)
    if _trace:
        _CACHE["last_exec_time_ns"] = res.exec_time_ns
        _CACHE["last_results"] = res
    return full


# revision 30
# speedup vs baseline: 1.1831x; 1.1831x over previous
"""FFM layer kernel for Trainium2 (8 NeuronCores, data-parallel over batch).

Math (reference):
  idx[b,j]  = 13 + j*10000 + sparse_x[b,j]                 (26 sparse fields)
  linear    = dense_x @ w[:13] + sum_j w[idx] + w0         (B,1)
  field_f   = einsum('bd,dfk', dense_x, v[:13]) + sum_j v[idx]   (B,39,8)
  s         = sum_f field_f                                 (B,8)
  cross     = 0.5*(sum_k s^2 - sum_{f,k} field_f^2)
  out       = sigmoid(linear + cross)

Device strategy (per core, 512 samples): fp16 padded table rows (384 f16 =
768B) carrying [v row | w | s_row[k]=sum_f v[.,f,k]], 768-idx two-field
gathers (int16 idx spans adjacent vocab blocks via +10000 offsets) balanced
3328 rows per SWDGE queue in 5 round-robin rounds (gen rate ~ drain rate,
quarters last), two interleaved fp16 accumulator chains on DVE plus an
fp32 chain for the high-magnitude s columns, PE matmul for the dense part,
DVE/ACT final phase with s read from the precomputed table columns.
"""

import os
import numpy as np

N_DENSE = 13
N_SPARSE = 26
VOCAB = 10000
N_FIELD = 39
N_FEAT = N_DENSE + N_SPARSE * VOCAB  # 260013
K = 8
ROW = N_FIELD * K  # 312 v elems; w at col 312; s_row at cols 313-320
USED = ROW + 1  # 313 cols accumulated in fp16 (v + w)
SCOL = ROW + 1  # first s column
ROWE = 384  # padded fp16 row -> 768 B (%256==0)
BATCH = 4096
N_CORES = 8
BC = BATCH // N_CORES  # 512 per core
P = 128
NCHUNK = BC // P  # 4
IDXC = BC // 16  # 32 int16 index columns per field
NQ = 4  # SWDGE queues / GPSIMD core pairs

# Per-queue gather schedule over the global idx sequence (field-major,
# position i = field*512 + sample).  Uniform 512-row single-field gathers:
# Q7 descriptor gen (~4.4us/round of 4) matches the HBM drain rate of a
# round (4*512*768B at ~360GB/s = 4.4us), so with single_packet doorbells
# (drain follows own gen) the rounds pipeline hole-free at the HBM roof.
# The two half-fields (6, 19) are 256-row gathers scheduled LAST so the
# final exposed drain is small.  Entry: (field_window_start, idx_lo, idx_hi).
SCHED = [
    [(0, 0, 768), (1, 768, 1536), (3, 1536, 2304), (4, 2304, 3072),
     (6, 3072, 3328)],
    [(7, 3584, 4352), (8, 4352, 5120), (10, 5120, 5888), (11, 5888, 6656),
     (6, 3328, 3584)],
    [(13, 6656, 7424), (14, 7424, 8192), (16, 8192, 8960), (17, 8960, 9728),
     (19, 9728, 9984)],
    [(20, 10240, 11008), (21, 11008, 11776), (23, 11776, 12544),
     (24, 12544, 13312), (19, 9984, 10240)],
]


def _segments(w, lo, hi):
    """Yield (tile_chunk0, acc_chunk0, n_chunks, field) per field segment."""
    segs = []
    f = lo // BC
    while f * BC < hi:
        seg_lo = max(lo, f * BC)
        seg_hi = min(hi, (f + 1) * BC)
        segs.append(((seg_lo - lo) // P, (seg_lo - f * BC) // P,
                     (seg_hi - seg_lo) // P, f))
        f += 1
    return segs

_CACHE: dict = {}


def _build_program():
    import concourse.bacc as bacc
    import concourse.tile as tile
    import concourse.mybir as mybir

    f32 = mybir.dt.float32
    f16 = mybir.dt.float16
    i16 = mybir.dt.int16

    nc = bacc.Bacc(
        "TRN2", target_bir_lowering=False, debug=False, num_swdge_queues=NQ
    )

    table = nc.dram_tensor("table", [N_FEAT, ROWE], f16, kind="ExternalInput")
    xt = nc.dram_tensor("xt", [P, BC], f32, kind="ExternalInput")
    vd = nc.dram_tensor("vd", [P, ROWE], f32, kind="ExternalInput")
    idx16 = nc.dram_tensor("idx16", [P, N_SPARSE * IDXC], i16, kind="ExternalInput")
    # out[p, c] = sigmoid result for sample c*128+p; host transposes
    out = nc.dram_tensor("out", [P, NCHUNK], f32, kind="ExternalOutput")

    with tile.TileContext(nc) as tc:
        with (
            tc.tile_pool(name="const", bufs=1) as cpool,
            tc.tile_pool(name="gather", bufs=4 * NQ) as gpool,
            tc.tile_pool(name="work", bufs=1) as wpool,
            tc.tile_pool(name="psum", bufs=1, space="PSUM") as ppool,
        ):
            # index tiles first: the gathers depend only on these
            idx_sb = cpool.tile([P, N_SPARSE * IDXC], i16)
            nc.scalar.dma_start(out=idx_sb[:], in_=idx16[:])
            # dense inputs on the other HWDGE queue; only the PE needs them
            xt_sb = cpool.tile([P, BC], f32)
            nc.sync.dma_start(out=xt_sb[:], in_=xt[:])
            vd_sb = cpool.tile([P, ROWE], f32)
            nc.sync.dma_start(out=vd_sb[:], in_=vd[:])

            # preload the ACT sigmoid table off the critical tail
            warm = cpool.tile([P, 1], f32)
            nc.vector.memset(warm[:], 0.0)
            warm2 = cpool.tile([P, 1], f32)
            nc.scalar.activation(
                warm2[:], warm[:], mybir.ActivationFunctionType.Sigmoid
            )

            # dense part: one matmul per chunk, each into its own PSUM bank
            psum = ppool.tile([P, NCHUNK, 512], f32, space="PSUM")
            for c in range(NCHUNK):
                nc.tensor.matmul(
                    out=psum[:, c, :ROWE],
                    lhsT=xt_sb[:, c * P:(c + 1) * P],
                    rhs=vd_sb[:],
                    start=True,
                    stop=True,
                )

            nregs = {}
            for q in range(NQ):
                for _w, lo, hi in SCHED[q]:
                    ni = hi - lo
                    if ni not in nregs:
                        nregs[ni] = nc.gpsimd.to_reg(ni)
            # emit gathers round-robin across queues; the first gather also
            # absorbs the one-time ~9us GPSIMD ext-isa IRAM load
            gathers = []  # (tile, queue, window_first, lo, hi)
            for r in range(max(len(s) for s in SCHED)):
                for q in range(NQ):
                    if r >= len(SCHED[q]):
                        continue
                    w, lo, hi = SCHED[q][r]
                    ni = hi - lo
                    nch = ni // P
                    last_f = (hi - 1) // BC
                    g = gpool.tile([P, nch, ROWE], f16, tag="g", name=f"g{q}_{r}")
                    base = N_DENSE + w * VOCAB
                    nc.gpsimd.dma_gather(
                        out_ap=g[:],
                        in_ap=table[base:base + (last_f - w + 1) * VOCAB, :],
                        idxs_ap=idx_sb[:, lo // 16:hi // 16],
                        num_idxs=ni,
                        num_idxs_reg=nregs[ni],
                        elem_size=ROWE,
                        single_packet=True,
                        queue_num=q,
                    )
                    gathers.append((g, q, w, lo, hi))

            # two fp16 accumulator chains with alternating segment assignment
            # (adjacent DVE adds hit different chains so they pipeline) plus
            # a flat fp32 chain for the high-magnitude s columns (313-320).
            # memset inits run during the startup dead time.
            accs = [wpool.tile([P, NCHUNK, USED + 1], f16, name=f"acc{i}")
                    for i in range(2)]
            sacc2 = wpool.tile([P, 2 * NCHUNK, K], f32)
            for a in accs:
                nc.vector.memset(a[:], 0.0)
            nc.vector.memset(sacc2[:], 0.0)

            nseg = 0
            for t, (g, q, w, lo, hi) in enumerate(gathers):
                for tc0, ac0, nch, f in _segments(w, lo, hi):
                    a = accs[nseg % 2][:, ac0:ac0 + nch, :]
                    nc.vector.tensor_tensor(
                        out=a, in0=a,
                        in1=g[:, tc0:tc0 + nch, :USED + 1],
                        op=mybir.AluOpType.add,
                    )
                    nseg += 1
                # one flat fp32 s-column add per gather: within a tile,
                # acc_chunk - tile_chunk == (lo/128) mod 4, so slot base
                # x = (lo/128)%4 keeps slot % 4 == chunk for every sample
                x = (lo // P) % NCHUNK
                nch_t = (hi - lo) // P
                sd = sacc2[:, x:x + nch_t, :]
                nc.vector.tensor_tensor(
                    out=sd, in0=sd, in1=g[:, :, SCOL:SCOL + K],
                    op=mybir.AluOpType.add,
                )

            # combine the two chains, then field = psum + acc
            nc.vector.tensor_tensor(out=accs[0][:], in0=accs[0][:],
                                    in1=accs[1][:], op=mybir.AluOpType.add)
            field = wpool.tile([P, NCHUNK, 320], f32)
            nc.vector.tensor_tensor(
                out=field[:, :, :USED], in0=psum[:, :, :USED],
                in1=accs[0][:, :, :USED], op=mybir.AluOpType.add,
            )

            # tq = [field^2 (312 cols) | -s^2 (8 cols)]: one 320-col reduce
            # then yields qs - ssum = -2*cross directly.  ACT squares while
            # the DVE runs the small s ops into the tail columns.
            tq = wpool.tile([P, NCHUNK, ROW + K], f32)
            nc.scalar.square(tq[:, :, :ROW], field[:, :, :ROW])

            # s = dense part (psum cols 313-320, from vd s columns) + both
            # halves of the flat fp32 s accumulator
            s_t = wpool.tile([P, NCHUNK, K], f32)
            nc.vector.tensor_tensor(
                out=s_t[:], in0=sacc2[:, 0:NCHUNK, :],
                in1=sacc2[:, NCHUNK:2 * NCHUNK, :], op=mybir.AluOpType.add,
            )
            s = wpool.tile([P, NCHUNK, K], f32)
            nc.vector.tensor_tensor(
                out=s[:], in0=psum[:, :, SCOL:SCOL + K], in1=s_t[:],
                op=mybir.AluOpType.add,
            )
            nc.vector.scalar_tensor_tensor(
                out=tq[:, :, ROW:ROW + K], in0=s[:], scalar=-1.0, in1=s[:],
                op0=mybir.AluOpType.mult, op1=mybir.AluOpType.mult,
            )

            nd = wpool.tile([P, NCHUNK, 1], f32)
            nc.vector.reduce_sum(out=nd[:], in_=tq[:], axis=mybir.AxisListType.X)

            # dd = 0.5*(ssum - qs) + linear = -0.5*nd + linear
            dd = wpool.tile([P, NCHUNK, 1], f32)
            nc.vector.scalar_tensor_tensor(
                out=dd[:], in0=nd[:], scalar=-0.5, in1=field[:, :, ROW:ROW + 1],
                op0=mybir.AluOpType.mult, op1=mybir.AluOpType.add,
            )
            oc = wpool.tile([P, NCHUNK], f32)
            nc.scalar.activation(
                oc[:], dd[:, :, 0], mybir.ActivationFunctionType.Sigmoid
            )
            nc.sync.dma_start(out=out[:], in_=oc[:])

    nc.compile()
    return nc


def _prep_inputs(dense_x, sparse_x, w0, w, v):
    table = np.zeros((N_FEAT, ROWE), dtype=np.float16)
    table[:, :ROW] = v.reshape(N_FEAT, ROW).astype(np.float16)
    table[:, ROW] = w[:, 0].astype(np.float16)
    # per-k field sums: s_row[i, k] = sum_f v[i, f, k]
    table[:, SCOL:SCOL + K] = v.sum(axis=1).astype(np.float16)

    vd = np.zeros((P, ROWE), dtype=np.float32)
    vd[:N_DENSE, :ROW] = v[:N_DENSE].reshape(N_DENSE, ROW)
    vd[:N_DENSE, ROW] = w[:N_DENSE, 0]
    vd[N_DENSE, ROW] = np.float32(w0[0])
    vd[:N_DENSE, SCOL:SCOL + K] = v[:N_DENSE].sum(axis=1)

    xt_full = np.zeros((P, BATCH), dtype=np.float32)
    xt_full[:N_DENSE] = dense_x.T
    xt_full[N_DENSE] = 1.0

    # per-column idx value offsets: global position i = col*16 + row lives in
    # field i//512; its gather window starts at field w -> offset (f-w)*VOCAB
    col_off = np.zeros(N_SPARSE * IDXC, dtype=np.int16)
    for qsched in SCHED:
        for w, lo, hi in qsched:
            for col in range(lo // 16, hi // 16):
                f = (col * 16) // BC
                col_off[col] = (f - w) * VOCAB

    in_maps = []
    for r in range(N_CORES):
        b0 = r * BC
        sp = sparse_x[b0:b0 + BC].astype(np.int16)  # values < 10000 fit
        idx16 = np.zeros((P, N_SPARSE * IDXC), dtype=np.int16)
        for j in range(N_SPARSE):
            blk = sp[:, j].reshape(IDXC, 16).T
            idx16[:, j * IDXC:(j + 1) * IDXC] = np.tile(blk, (P // 16, 1))
        idx16 += col_off[None, :]
        in_maps.append(
            {
                "table": table,
                "xt": np.ascontiguousarray(xt_full[:, b0:b0 + BC]),
                "vd": vd,
                "idx16": idx16,
            }
        )
    return in_maps


def kernel(dense_x, sparse_x, w0, w, v, _trace=False, _trace_kwargs=None):
    from concourse.bass_utils import run_bass_kernel_spmd

    if "nc" not in _CACHE:
        _CACHE["nc"] = _build_program()
    nc = _CACHE["nc"]

    in_maps = _prep_inputs(dense_x, sparse_x, w0, w, v)
    kw = {}
    if _trace:
        kw["trace"] = True
        if _trace_kwargs:
            kw.update(_trace_kwargs)
    res = run_bass_kernel_spmd(nc, in_maps, core_ids=list(range(N_CORES)), **kw)
    # device out[p, c] holds sample c*128+p of the core's 512-sample slice
    outs = [res.results[r]["out"].T.reshape(BC, 1) for r in range(N_CORES)]
    full = np.concatenate(outs, axis=0).astype(np.float32)
    if _trace:
        _CACHE["last_exec_time_ns"] = res.exec_time_ns
        _CACHE["last_results"] = res
    return full
